# revision 41
# baseline (speedup 1.0000x reference)
"""Chamfer distance kernel for Trainium2 (8 NeuronCores, batch-parallel).

Problem: input1 (8,4096,3), input2 (8,4096,3) fp32.
  D[b,n,m] = ||input1[b,n]-input2[b,m]||
  loss = mean_b( mean_m min_n D + mean_n min_m D )

Per core (one batch): -D2 = 2*x1.x2 - n1[n] - n2[m] computed on the PE as a
single K=13 float32r matmul whose contraction rows carry the hi/lo split of
the coordinates plus the hi/lo split of both squared norms (the hi+lo f32r
pair reconstructs fp32 exactly, so D2 is fp32-accurate up to the dropped
lo*lo term ~2^-26). The sign is flipped so both reductions are MAX.

Main loop, per 128x2048 PSUM group: the fp32 PSUM tile is converted once to
bf16 SBUF by the ACT engine (the binding engine: 64 x ~1.9us); the
column-min accumulates with a bf16 tensor_tensor max on DVE (2x mode, vs a
ping-pong partner so it is never in-place); the row-min comes from ONE DVE
tensor_scalar whose accum_out row-reduces with op1=max (the only DVE op
that both reduces and runs in 4x perf mode). Real-HW constraints found the
hard way: gpsimd cannot access PSUM, cannot run generic tensor ops at all
(neuronxcc rejects both), and DMA CCE supports add but not max — so the
reduction has exactly two usable engines, ACT + DVE.

Setup: each side builds a [128, 416] per-point tile (hi limbs pre-rounded to
f32r via copy, lo limbs exact, norms, consts) in 13 column sections ordered
like the L/R staging rows, bounces it to DRAM, and lands it in the [13, 4096]
matmul operand with two strided DMAs (descriptor-fixed HWDGE cost makes DMA
count, not bytes, the driver). Tail: cmb is dumped raw in two column halves
(the first overlaps the last tiles) and the host does the 128-way partition
max plus clamp/sqrt/mean (the batch mean is the unshard step).
"""

import sys

sys.path.insert(0, "/opt/trn_rl_repo")

import numpy as np
from contextlib import ExitStack

import concourse.bacc as bacc
import concourse.tile as tile
import concourse.bass_isa as bass_isa
from concourse import mybir
from concourse.bass_utils import run_bass_kernel_spmd

B, NPTS, KDIM = 8, 4096, 3
IT_N = NPTS // 128   # 32 I-tiles of 128 rows (x1 points)
JC_N = NPTS // 512   # 8 J-chunks of 512 cols (x2 points)

F32 = mybir.dt.float32
F32R = mybir.dt.float32r

_cached = {}


def _build(reps: int = 1, loop_n: int = 1, GSPAN: int = 2048, PSB: int = 2, CBB: int = 3, CA: int = 2048, NWARM: int = 30):
    nc = bacc.Bacc("TRN2", target_bir_lowering=False, debug=False, num_devices=B)

    BF16 = mybir.dt.bfloat16
    KROWS = 13
    SECW = KROWS * 32  # 416: one 32-col section per staging row

    x1_d = nc.dram_tensor("x1", [NPTS, KDIM], F32, kind="ExternalInput").ap()
    x2_d = nc.dram_tensor("x2", [NPTS, KDIM], F32, kind="ExternalInput").ap()
    outc_d = nc.dram_tensor("outc", [128, NPTS], BF16, kind="ExternalOutput").ap()
    outr_d = nc.dram_tensor("outr", [128, IT_N], F32, kind="ExternalOutput").ap()
    scr1_d = nc.dram_tensor("scr1", [128 * SECW], F32, kind="Internal").ap()
    scr2_d = nc.dram_tensor("scr2", [128 * SECW], F32, kind="Internal").ap()

    MX = mybir.AluOpType.max
    MUL = mybir.AluOpType.mult
    X = mybir.AxisListType.X

    with tile.TileContext(nc) as tc, ExitStack() as ctx:
        sb = ctx.enter_context(tc.tile_pool(name="sb", bufs=1))
        scr = ctx.enter_context(tc.tile_pool(name="scr", bufs=2))
        cbp = ctx.enter_context(tc.tile_pool(name="cbp", bufs=CBB))
        tsp = ctx.enter_context(tc.tile_pool(name="tsp", bufs=2))
        ps = ctx.enter_context(tc.tile_pool(name="ps", bufs=PSB, space="PSUM"))

        # P = sum_r L[r]*R[r] = 2*x1.x2 - n1 - n2 = -D2 (float32r limbs:
        # hi+lo reconstructs fp32 exactly, so D2 is fp32-accurate up to the
        # dropped lo*lo term ~2^-26):
        # r    L row         R row
        # 0-2  x1hi          2*x2hi
        # 3-5  x1hi          2*x2lo
        # 6-8  x1lo          2*x2hi
        # 9    n1hi          -1
        # 10   n1lo          -1
        # 11   +1            -n2hi
        # 12   +1            -n2lo
        L = sb.tile([KROWS, NPTS], F32R)
        R = sb.tile([KROWS, NPTS], F32R)

        # PE p-state warm-up: dependency-free dummy matmuls keep the tensor
        # engine busy through the setup (>3us of continuous execution) so the
        # first real matmuls start at the full 2.4 GHz clock instead of
        # ramping through the low/mid p-states during pipeline fill
        if NWARM:
            dum_l = sb.tile([1, 16], F32R)
            dum_r = sb.tile([1, 512], F32R)
            nc.vector.memset(dum_l[:].bitcast(F32), 0.0)
            nc.vector.memset(dum_r[:].bitcast(F32), 0.0)
            for _ in range(NWARM):
                Pd = ps.tile([128, GSPAN], F32, tag="P")
                nc.tensor.matmul(
                    Pd[0:1, 0:512], dum_l[:, 0:1], dum_r[:], start=True, stop=True
                )

        # Per-point math runs in natural layout (128 partitions x 32 points)
        # so every DVE lane works. comb's 13 column sections mirror the L/R
        # rows; section s lands in row s via the single strided DMA below.
        # Column order of L/R is point index n = p*32 + t everywhere.
        def stage_side(x_d, scale, norm_factor, hi_secs, lo_sec, nhi_sec,
                       const_sec, const_val, scratch_d, T, dma, use_act):
            # use_act: run the unary setup ops on ACT (x2 side) so the two
            # sides' chains execute in parallel instead of queueing on DVE
            def mul_(out, in_, s):
                nc.scalar.mul(out, in_, s) if use_act else nc.vector.tensor_scalar_mul(out, in_, s)

            def copy_(out, in_):
                nc.scalar.copy(out, in_) if use_act else nc.vector.tensor_copy(out, in_)

            def square_(out, in_):
                nc.scalar.square(out, in_) if use_act else nc.vector.tensor_tensor(out, in_, in_, op=MUL)

            xn = scr.tile([128, 96], F32, tag="nat")
            dma.dma_start(xn[:], x_d.rearrange("(p t) k -> p (t k)", p=128))
            # de-interleave (t k) -> (k t), fusing the *2 scale for x2
            xsep = scr.tile([128, 96], F32, tag="natsep")
            osep = xsep[:].rearrange("p (k t) -> p k t", t=32)
            isep = xn[:].rearrange("p (t k) -> p k t", k=KDIM)
            if scale != 1.0:
                mul_(osep, isep, scale)
            else:
                copy_(osep, isep)
            comb = scr.tile([128, SECW], F32, tag="comb")
            # hi limbs: the f32r-typed copy rounds; the stored bits are both
            # valid f32 and exactly what the PE reads as f32r
            for s in hi_secs:
                copy_(comb[:, s * 32 : (s + 3) * 32].bitcast(F32R), xsep[:])
            h0 = hi_secs[0] * 32
            nc.vector.tensor_sub(
                comb[:, lo_sec * 32 : (lo_sec + 3) * 32],
                xsep[:],
                comb[:, h0 : h0 + 96],
            )
            # norms (of the scaled coords), then hi/lo split
            sq = scr.tile([128, 96], F32, tag="natsq")
            square_(sq[:], xsep[:])
            nnr = scr.tile([128, 32], F32, tag="natn")
            nc.vector.tensor_reduce(
                nnr[:], sq[:].rearrange("p (k t) -> p t k", t=32), axis=X,
                op=mybir.AluOpType.add,
            )
            f = norm_factor / (scale * scale)
            if f != 1.0:
                mul_(nnr[:], nnr[:], f)
            nh0 = nhi_sec * 32
            copy_(comb[:, nh0 : nh0 + 32].bitcast(F32R), nnr[:])
            nc.vector.tensor_sub(
                comb[:, nh0 + 32 : nh0 + 64], nnr[:], comb[:, nh0 : nh0 + 32]
            )
            nc.vector.memset(comb[:, const_sec * 32 : (const_sec + 2) * 32], const_val)
            # bounce to DRAM, then land all 13 rows in TWO strided DMAs (same
            # queue, so reads order after the write; the first covers the
            # leading columns so the first matmuls start sooner)
            dma.dma_start(scratch_d.rearrange("(p c) -> p c", p=128), comb[:])
            sv = scratch_d.rearrange("(p r t) -> r p t", p=128, r=KROWS)
            tv = T[0:KROWS, :].bitcast(F32).rearrange("r (p t) -> r p t", p=128)
            psplit = 64
            dma.dma_start(tv[:, 0:psplit, :], sv[:, 0:psplit, :])
            dma.dma_start(tv[:, psplit:, :], sv[:, psplit:, :])

        # L: hi(0-2), hi(3-5), lo(6-8), n1hi(9), n1lo(10), +1(11,12)
        stage_side(x1_d, 1.0, 1.0, (0, 3), 6, 9, 11, 1.0, scr1_d, L, nc.sync, False)
        # R: 2x2hi(0-2), 2x2lo(3-5), 2x2hi(6-8), -1(9,10), n2hi(11), n2lo(12)
        stage_side(x2_d, 2.0, -1.0, (0, 6), 3, 11, 9, -1.0, scr2_d, R, nc.scalar, False)

        # ping-pong accumulators: out != in0 keeps the bf16 tensor_tensor in
        # its 2x perf mode (in-place aliasing falls back to 1x)
        cmb_a = sb.tile([128, NPTS], BF16)
        cmb_b = sb.tile([128, NPTS], BF16)
        nc.gpsimd.memset(cmb_a[:], -3.0e38)
        rmall = sb.tile([128, IT_N], F32)

        GRP = GSPAN // 512  # jc chunks per PSUM group
        NG = JC_N // GRP   # groups per I-tile
        # one extra column for the split final group's second rowmax accum
        rg_all = sb.tile([128, IT_N * NG + 1], F32)
        import contextlib
        loop_ctx = tc.For_i(0, loop_n, 1) if loop_n > 1 else contextlib.nullcontext()
        # Real-HW engine constraints (neuronxcc BIR verifier): gpsimd cannot
        # access PSUM and cannot run generic tensor ops at all, so the whole
        # reduction lives on ACT (conversion) + DVE (colmax 2x + rowmax 4x).
        with loop_ctx:
          for _rep in range(reps):
            for it in range(IT_N):
                for g in range(NG):
                    P = ps.tile([128, GSPAN], F32, tag="P")
                    for j in range(GRP):
                        nc.tensor.matmul(
                            P[:, j * 512 : (j + 1) * 512],
                            L[:, it * 128 : (it + 1) * 128],
                            R[:, (g * GRP + j) * 512 : (g * GRP + j + 1) * 512],
                            start=True,
                            stop=True,
                        )
                    src, dst = (cmb_a, cmb_b) if it % 2 == 0 else (cmb_b, cmb_a)
                    g0 = g * GSPAN
                    C = cbp.tile([128, GSPAN], BF16)
                    # the FINAL group runs in two column halves so the tail's
                    # serial conv -> colmax -> dump drain chain is half as deep
                    last = it == IT_N - 1 and g == NG - 1 and _rep == reps - 1
                    Hh = GSPAN // 2
                    chunks = ((0, Hh), (Hh, GSPAN)) if last else ((0, GSPAN),)
                    for ci, (c0, c1) in enumerate(chunks):
                        # PSUM->SBUF bf16 conversion (ACT, the binding
                        # engine) — except the very first group, which DVE
                        # converts (it idles during pipeline fill anyway) so
                        # two conversions run concurrently at the start
                        if it == 0 and g == 0 and _rep == 0:
                            nc.vector.tensor_copy(C[:, c0:c1], P[:, c0:c1])
                        else:
                            nc.scalar.copy(C[:, c0:c1], P[:, c0:c1])
                        # colmax accumulate (bf16 tensor_tensor, 2x mode)
                        nc.vector.tensor_tensor(
                            dst[:, g0 + c0 : g0 + c1],
                            src[:, g0 + c0 : g0 + c1],
                            C[:, c0:c1],
                            op=MX,
                        )
                        # rowmax: ONE 4x tensor_scalar pass, accum_out
                        # reduces the row with op1=max
                        ts = tsp.tile([128, GSPAN], BF16)
                        col = it * NG + g if ci == 0 else IT_N * NG
                        nc.vector.tensor_scalar(
                            ts[:, c0:c1], C[:, c0:c1], -3.0e38, None,
                            op0=MX, op1=MX,
                            accum_out=rg_all[:, col : col + 1],
                        )

        # fold the NG per-group rowmaxes into one column per I-tile (plus the
        # split final group's extra accum column into its tile's slot)
        rga_v = rg_all[:, 0 : IT_N * NG].rearrange("p (t g) -> p t g", g=NG)
        nc.vector.tensor_tensor(rmall[:], rga_v[:, :, 0], rga_v[:, :, 1], op=MX)
        for g in range(2, NG):
            nc.vector.tensor_tensor(rmall[:], rmall[:], rga_v[:, :, g], op=MX)
        nc.vector.tensor_tensor(
            rmall[:, IT_N - 1 : IT_N],
            rmall[:, IT_N - 1 : IT_N],
            rg_all[:, IT_N * NG : IT_N * NG + 1],
            op=MX,
        )

        # ---- tail: dump cmb raw in two column halves (the first only needs
        # the g=0 groups, so it overlaps the last tiles); the host does the
        # 128-way partition max plus clamp/sqrt/mean ----
        cmb_fin = cmb_b if (IT_N * reps) % 2 == 1 else cmb_a
        H = NPTS // 2
        Q = NPTS - GSPAN // 2  # boundary of the split final group's 2nd half
        nc.sync.dma_start(outc_d[:, 0:H], cmb_fin[:, 0:H])
        nc.sync.dma_start(outc_d[:, H:Q], cmb_fin[:, H:Q])
        nc.sync.dma_start(outc_d[:, Q:], cmb_fin[:, Q:])
        nc.scalar.dma_start(outr_d, rmall[:])

    nc.compile()
    return nc


def _get(reps: int = 1, loop_n: int = 1, **kw):
    key = (reps, loop_n, tuple(sorted(kw.items())))
    if key not in _cached:
        _cached[key] = _build(reps, loop_n, **kw)
    return _cached[key]


def kernel(input1: np.ndarray, input2: np.ndarray, _trace: bool = False):
    nc = _get()
    input1 = np.ascontiguousarray(np.asarray(input1, dtype=np.float32))
    input2 = np.ascontiguousarray(np.asarray(input2, dtype=np.float32))
    in_maps = [{"x1": input1[b], "x2": input2[b]} for b in range(B)]
    res = run_bass_kernel_spmd(nc, in_maps, core_ids=list(range(B)), trace=_trace)
    losses = []
    for b in range(B):
        r = res.results[b]
        c = -np.asarray(r["outc"], dtype=np.float64).max(axis=0)
        rr = -np.asarray(r["outr"], dtype=np.float64).reshape(-1)
        d0 = np.sqrt(np.clip(c, 0.0, None))
        d1 = np.sqrt(np.clip(rr, 0.0, None))
        losses.append(d0.mean() + d1.mean())
    out = np.float32(np.mean(losses))
    if _trace:
        return out, res
    return out
